# revision 1
# baseline (speedup 1.0000x reference)
"""Multi-head differential attention (full-width) on 8 Trainium2 NeuronCores.

Contract: kernel(**inputs) takes the FULL inputs of reference.setup_inputs()
and returns the FULL [8, 512, 8192] float32 output.

Strategy: pure data-parallel over batch — one batch element per NeuronCore.
Per core, a single fused Bass/Tile kernel computes:
  - qT/kT projections o-block by o-block (float32r matmuls at full PE rate),
    with the score matmuls fused into the same loop so scores finish when the
    projections do (no qT/kT ever stored: only a rotating 2-tile window),
  - softmax via ScalarE Exp with accum_out row-sums (scores are bounded ~±25,
    so no max-subtraction is needed in fp32),
  - differential combine P = e1/d1 - lam ⊙ e2/d2 on VectorE,
  - PE-transpose of P, then a fused v-projection + attention-V matmul loop
    that streams Wv and writes each 512-column output tile as it completes.

Weights are repacked host-side (pure layout permutation + no dtype change) so
every DMA lands with long contiguous per-partition runs.
"""
import ml_dtypes
import numpy as np
from contextlib import ExitStack

import concourse.bass as bass
import concourse.mybir as mybir
import concourse.tile as tile
from concourse.bass_utils import run_bass_kernel_spmd
from concourse.masks import make_identity

F32 = mybir.dt.float32
F32R = mybir.dt.float32r
BF16 = mybir.dt.bfloat16
P = 128
B = 8
S = 512          # sequence length (= d_head for the lambda broadcast)
DM = 4096        # model dim (projection contraction)
DH = 4096        # width of each q/k half (attention contraction)
D2 = 2 * DH      # projection output width
NQB = S // P     # 4 query blocks
NT = DM // P     # 32 contraction tiles
NOB = D2 // P    # 64 projection o-blocks
HOB = DH // P    # 32 o-blocks per half
NODT = D2 // 512  # 16 output column tiles
WV_CH = 8        # dq-tiles per streamed Wv chunk
NCH = NT // WV_CH
SCALE = float(1.0 / np.sqrt(512.0))

MAX_WAITS = 1  # this walrus build allows one sync-wait per instruction


def _split_sync_waits(nc):
    """Move excess per-instruction sync waits onto preceding no-ops (same
    engine, program order) — semantically identical, keeps walrus happy."""
    for f in nc.m.functions:
        for bb in f.blocks:
            new_insts = []
            for inst in bb.instructions:
                si = inst.sync_info
                if si is not None and si.on_wait and len(si.on_wait) > MAX_WAITS:
                    waits = list(si.on_wait)
                    excess, keep = waits[MAX_WAITS:], waits[:MAX_WAITS]
                    for ci in range(0, len(excess), MAX_WAITS):
                        new_insts.append(mybir.InstNoOp(
                            name=f"{inst.name}-waitsplit{ci}",
                            engine=inst.engine, ins=[], outs=[],
                            sync_info=mybir.SyncInfo(
                                on_wait=excess[ci:ci + MAX_WAITS], on_update=[]),
                            text_hint="waitsplit"))
                    si.on_wait = keep
                new_insts.append(inst)
            bb.instructions = new_insts


def build_nc():
    nc = bass.Bass()
    xT = nc.declare_dram_parameter("xT", [DM, S], F32R, isOutput=False)
    wq = nc.declare_dram_parameter("wq", [NOB, P, NT, P], F32R, isOutput=False)
    wk = nc.declare_dram_parameter("wk", [NOB, P, NT, P], F32R, isOutput=False)
    wv = nc.declare_dram_parameter("wv", [NODT, P, NT, 512], BF16, isOutput=False)
    qb = nc.declare_dram_parameter("qb", [P, NOB], F32, isOutput=False)
    kb = nc.declare_dram_parameter("kb", [P, NOB], F32, isOutput=False)
    vb = nc.declare_dram_parameter("vb", [D2], BF16, isOutput=False)
    lam = nc.declare_dram_parameter("lam", [S], F32R, isOutput=False)
    ones = nc.declare_dram_parameter("ones", [P], F32R, isOutput=False)
    onesb = nc.declare_dram_parameter("onesb", [P], BF16, isOutput=False)
    xTb = nc.declare_dram_parameter("xTb", [DM, S], BF16, isOutput=False)
    out = nc.declare_dram_parameter("out", [S, D2], F32, isOutput=True)

    with tile.TileContext(nc) as tc, ExitStack() as top:
        const = top.enter_context(tc.tile_pool(name="const", bufs=1))
        xT_sb = const.tile([P, NT, S], F32R, name="xT_sb")
        xT_r = xT.rearrange("(t p) s -> p t s", p=P)
        for xc in range(4):
            nc.sync.dma_start(xT_sb[:, xc * (NT // 4):(xc + 1) * (NT // 4), :],
                              xT_r[:, xc * (NT // 4):(xc + 1) * (NT // 4), :])
        qb_sb = const.tile([P, NOB], F32, name="qb_sb")
        nc.sync.dma_start(qb_sb[:], qb[:])
        kb_sb = const.tile([P, NOB], F32, name="kb_sb")
        nc.sync.dma_start(kb_sb[:], kb[:])
        lam_sb = const.tile([1, S], F32R, name="lam_sb")
        nc.sync.dma_start(lam_sb[:], lam[None, :])
        ones_row = const.tile([1, P], F32R, name="ones_row")
        nc.sync.dma_start(ones_row[:], ones[None, :])
        onesb_row = const.tile([1, P], BF16, name="onesb_row")
        nc.sync.dma_start(onesb_row[:], onesb[None, :])
        xTb_sb = const.tile([P, NT, S], BF16, name="xTb_sb")
        ident = const.tile([P, P], F32, name="ident")
        make_identity(nc, ident[:])

        # lam broadcast to all 128 partitions via K=1 matmul
        lam_bc = const.tile([P, S], F32, name="lam_bc")
        with tc.tile_pool(name="ps_misc", bufs=1, space="PSUM") as ps_misc:
            pt = ps_misc.tile([P, 512], F32, name="lam_ps")
            nc.tensor.matmul(pt[:], ones_row[:], lam_sb[:], start=True, stop=True)
            nc.vector.tensor_copy(out=lam_bc[:], in_=pt[:])

        e_sb = const.tile([P, 2, NQB, S], F32, name="e_sb")
        d_sb = const.tile([P, 2, NQB], F32, name="d_sb")
        r_sb = const.tile([P, 2, NQB], F32, name="r_sb")
        P_sb = const.tile([P, NQB, S], F32, name="P_sb")
        PT_sb = const.tile([P, S // P, S], F32R, name="PT_sb")

        # ---- Phase A: fused q/k projections + score accumulation ----
        with ExitStack() as phA:
            wqk = phA.enter_context(tc.tile_pool(name="wqk", bufs=5))
            qksb = phA.enter_context(tc.tile_pool(name="qksb", bufs=6))
            ps_proj = phA.enter_context(tc.tile_pool(name="ps_proj", bufs=3, space="PSUM"))
            ps_scores = phA.enter_context(tc.tile_pool(name="ps_scores", bufs=5, space="PSUM"))

            for h in range(2):
                sc_tiles = [ps_scores.tile([P, S], F32, name=f"sc_{h}_{qbk}", tag="sc")
                            for qbk in range(NQB)]
                prev_qk = None

                def emit_scores(i, q_sb, k_sb):
                    for qbk in range(NQB):
                        nc.tensor.matmul(sc_tiles[qbk][:],
                                         q_sb[:, qbk * P:(qbk + 1) * P], k_sb[:],
                                         start=(i == 0), stop=(i == HOB - 1))

                for i in range(HOB):
                    ob = h * HOB + i
                    if h == 1 and i == HOB - 8:
                        nc.sync.dma_start(xTb_sb[:], xTb.rearrange("(t p) s -> p t s", p=P))
                    pq = ps_proj.tile([P, S], F32, name="pq", tag="pp")
                    for cw in range(2):
                        wt = wqk.tile([P, NT // 2, P], F32R, name="wt_q", tag="w")
                        nc.sync.dma_start(wt[:], wq[ob][:, cw * (NT // 2):(cw + 1) * (NT // 2), :])
                        for tt in range(NT // 2):
                            t = cw * (NT // 2) + tt
                            nc.tensor.matmul(pq[:], wt[:, tt, :], xT_sb[:, t, :],
                                             start=(t == 0), stop=(t == NT - 1))
                    q_sb = qksb.tile([P, S], F32R, name="q_sb", tag="qk")
                    nc.vector.tensor_scalar(q_sb[:], pq[:], qb_sb[:, ob:ob + 1], SCALE,
                                            mybir.AluOpType.add, mybir.AluOpType.mult)
                    pk = ps_proj.tile([P, S], F32, name="pk", tag="pp")
                    for cw in range(2):
                        wtk = wqk.tile([P, NT // 2, P], F32R, name="wt_k", tag="w")
                        nc.sync.dma_start(wtk[:], wk[ob][:, cw * (NT // 2):(cw + 1) * (NT // 2), :])
                        for tt in range(NT // 2):
                            t = cw * (NT // 2) + tt
                            nc.tensor.matmul(pk[:], wtk[:, tt, :], xT_sb[:, t, :],
                                             start=(t == 0), stop=(t == NT - 1))
                    if prev_qk is not None:
                        emit_scores(i - 1, *prev_qk)
                    k_sb = qksb.tile([P, S], F32R, name="k_sb", tag="qk")
                    nc.vector.tensor_scalar(k_sb[:], pk[:], kb_sb[:, ob:ob + 1], None,
                                            mybir.AluOpType.add)
                    prev_qk = (q_sb, k_sb)
                emit_scores(HOB - 1, *prev_qk)
                for qbk in range(NQB):
                    nc.scalar.activation(e_sb[:, h, qbk, :], sc_tiles[qbk][:],
                                         mybir.ActivationFunctionType.Exp,
                                         accum_out=d_sb[:, h, qbk:qbk + 1])

        # ---- Phase B+C: combine + transpose overlapped with v projection/AV ----
        nc.vector.reciprocal(r_sb[:, :, :], d_sb[:, :, :])
        with ExitStack() as phC:
            cmb = phC.enter_context(tc.tile_pool(name="cmb", bufs=2))
            wvp = phC.enter_context(tc.tile_pool(name="wvp", bufs=3))
            vbp = phC.enter_context(tc.tile_pool(name="vbp", bufs=2))
            vsb = phC.enter_context(tc.tile_pool(name="vsb", bufs=2))
            osb = phC.enter_context(tc.tile_pool(name="osb", bufs=4))
            ps_tr = phC.enter_context(tc.tile_pool(name="ps_tr", bufs=2, space="PSUM"))
            ps_vp = phC.enter_context(tc.tile_pool(name="ps_vp", bufs=4, space="PSUM"))
            ps_av = phC.enter_context(tc.tile_pool(name="ps_av", bufs=2, space="PSUM"))

            def combine_and_transpose():
                for qbk in range(NQB):
                    tmp = cmb.tile([P, S], F32, name="tmp", tag="tmp")
                    nc.vector.tensor_tensor(tmp[:], e_sb[:, 1, qbk, :], lam_bc[:],
                                            mybir.AluOpType.mult)
                    nc.vector.tensor_scalar(tmp[:], tmp[:], r_sb[:, 1, qbk:qbk + 1], None,
                                            mybir.AluOpType.mult)
                    nc.vector.tensor_scalar(P_sb[:, qbk, :], e_sb[:, 0, qbk, :],
                                            r_sb[:, 0, qbk:qbk + 1], None,
                                            mybir.AluOpType.mult)
                    nc.vector.tensor_tensor(P_sb[:, qbk, :], P_sb[:, qbk, :], tmp[:],
                                            mybir.AluOpType.subtract)
                for qbk in range(NQB):
                    for kbk in range(S // P):
                        pt2 = ps_tr.tile([P, P], F32, name="pt2", tag="pt")
                        nc.tensor.transpose(pt2[:], P_sb[:, qbk, kbk * P:(kbk + 1) * P],
                                            ident[:])
                        nc.vector.tensor_copy(out=PT_sb[:, kbk, qbk * P:(qbk + 1) * P],
                                              in_=pt2[:])

            v_tiles = [None] * NODT

            def do_av(odt):
                for qbk in range(NQB):
                    pav = ps_av.tile([P, 512], F32, name="pav", tag="av")
                    for kbk in range(S // P):
                        nc.tensor.matmul(pav[:],
                                         PT_sb[:, kbk, qbk * P:(qbk + 1) * P],
                                         v_tiles[odt][:, kbk, :],
                                         start=(kbk == 0), stop=(kbk == S // P - 1))
                    o_st = osb.tile([P, 512], F32, name="o_st", tag="o")
                    nc.vector.tensor_copy(out=o_st[:], in_=pav[:])
                    nc.sync.dma_start(out[qbk * P:(qbk + 1) * P, odt * 512:(odt + 1) * 512],
                                      o_st[:])

            for odt in range(NODT):
                pv = [ps_vp.tile([P, 512], F32, name=f"pv{sb}", tag="vp")
                      for sb in range(NQB)]
                vbt = vbp.tile([1, 512], BF16, name="vbt", tag="vb")
                nc.sync.dma_start(vbt[:], vb[None, odt * 512:(odt + 1) * 512])
                for sb in range(NQB):
                    nc.tensor.matmul(pv[sb][:], onesb_row[:], vbt[:],
                                     start=True, stop=False)
                for c in range(NCH):
                    wvt = wvp.tile([P, WV_CH, 512], BF16, name="wvt", tag="wv")
                    nc.sync.dma_start(wvt[:], wv[odt][:, c * WV_CH:(c + 1) * WV_CH, :])
                    for sb in range(NQB):
                        for tt in range(WV_CH):
                            t = c * WV_CH + tt
                            nc.tensor.matmul(pv[sb][:], xTb_sb[:, t, sb * P:(sb + 1) * P],
                                             wvt[:, tt, :],
                                             start=False, stop=(t == NT - 1))
                v_t = vsb.tile([P, S // P, 512], F32R, name="v_t", tag="v")
                for sb in range(NQB):
                    nc.vector.tensor_copy(out=v_t[:, sb, :], in_=pv[sb][:])
                v_tiles[odt] = v_t
                if odt == 0:
                    # PE chews on v-proj(0) while DVE does the combine and the
                    # transposes queue up behind it — hides the softmax tail.
                    combine_and_transpose()
                if odt >= 1:
                    do_av(odt - 1)
                    v_tiles[odt - 1] = None
            do_av(NODT - 1)

    _split_sync_waits(nc)
    return nc


def pack_shared(wq_w, wq_b, wk_w, wk_b, wv_w, wv_b,
                lambda_q1, lambda_k1, lambda_q2, lambda_k2):
    lam = (np.exp(lambda_q1 * lambda_k1) - np.exp(lambda_q2 * lambda_k2)
           + np.float32(0.8)).astype(np.float32)
    return {
        "wq": np.ascontiguousarray(wq_w.reshape(NOB, P, NT, P).transpose(0, 3, 2, 1)),
        "wk": np.ascontiguousarray(wk_w.reshape(NOB, P, NT, P).transpose(0, 3, 2, 1)),
        "wv": np.ascontiguousarray(wv_w.reshape(NODT, 512, NT, P).transpose(0, 3, 2, 1)).astype(ml_dtypes.bfloat16),
        "qb": np.ascontiguousarray(wq_b.reshape(NOB, P).T),
        "kb": np.ascontiguousarray(wk_b.reshape(NOB, P).T),
        "vb": np.ascontiguousarray(wv_b).astype(ml_dtypes.bfloat16),
        "lam": lam,
        "ones": np.ones(P, np.float32),
        "onesb": np.ones(P, ml_dtypes.bfloat16),
    }


def make_in_maps(x, wq_w, wq_b, wk_w, wk_b, wv_w, wv_b,
                 lambda_q1, lambda_k1, lambda_q2, lambda_k2):
    shared = pack_shared(wq_w, wq_b, wk_w, wk_b, wv_w, wv_b,
                         lambda_q1, lambda_k1, lambda_q2, lambda_k2)
    return [{**shared, "xT": np.ascontiguousarray(x[b].T),
         "xTb": np.ascontiguousarray(x[b].T).astype(ml_dtypes.bfloat16)} for b in range(B)]


_NC_CACHE = None


def get_nc():
    global _NC_CACHE
    if _NC_CACHE is None:
        _NC_CACHE = build_nc()
    return _NC_CACHE


def kernel(x, wq_w, wq_b, wk_w, wk_b, wv_w, wv_b,
           lambda_q1, lambda_k1, lambda_q2, lambda_k2):
    args = [np.asarray(a, dtype=np.float32) for a in
            (x, wq_w, wq_b, wk_w, wk_b, wv_w, wv_b,
             lambda_q1, lambda_k1, lambda_q2, lambda_k2)]
    nc = get_nc()
    in_maps = make_in_maps(*args)
    res = run_bass_kernel_spmd(nc, in_maps, list(range(B)))
    return np.stack([res.results[b]["out"] for b in range(B)]).astype(np.float32)



# revision 3
# speedup vs baseline: 1.0768x; 1.0768x over previous
"""Multi-head differential attention (full-width) on 8 Trainium2 NeuronCores.

Contract: kernel(**inputs) takes the FULL inputs of reference.setup_inputs()
and returns the FULL [8, 512, 8192] float32 output.

Strategy: pure data-parallel over batch — one batch element per NeuronCore.
Per core, a single fused Bass/Tile kernel computes:
  - qT/kT projections o-block by o-block in bf16 (FWL-rate LDWEIGHTS: the
    fp32 weight path costs +20ns on every matmul because Fast Weight Load
    requires a non-fp32 stationary operand), with the score matmuls fused
    into the same loop (no qT/kT ever stored: only a rotating 2-tile window),
  - softmax via ScalarE Exp with accum_out row-sums (scores are bounded ~±25,
    so no max-subtraction is needed in fp32),
  - differential combine P = e1/d1 - lam ⊙ e2/d2 on VectorE,
  - PE-transpose of P, then a fused v-projection + attention-V matmul loop
    that streams Wv and writes each 512-column output tile as it completes.

Startup: the PE is cold (1.2 GHz HAM throttle) for its first ~3.4us and the
first x chunks take ~15us to land, so the kernel front-loads useful/dummy
matmuls with no DMA dependencies (v-bias broadcast + identity multiplies) to
warm the clock and hide the initial DMA wait. The first Wv chunk is
prefetched near the end of phase A so the PE never idles at the phase
boundary (an idle >3.4us would also re-throttle the clock).

Weights are repacked host-side (layout permutation + bf16 cast) so every DMA
lands with long contiguous per-partition runs.
"""
import ml_dtypes
import numpy as np
from contextlib import ExitStack

import concourse.bass as bass
import concourse.mybir as mybir
import concourse.tile as tile
from concourse.bass_utils import run_bass_kernel_spmd
from concourse.masks import make_identity

F32 = mybir.dt.float32
F32R = mybir.dt.float32r
BF16 = mybir.dt.bfloat16
P = 128
B = 8
S = 512          # sequence length (= d_head for the lambda broadcast)
DM = 4096        # model dim (projection contraction)
DH = 4096        # width of each q/k half (attention contraction)
D2 = 2 * DH      # projection output width
NQB = S // P     # 4 query blocks
NT = DM // P     # 32 contraction tiles
NOB = D2 // P    # 64 projection o-blocks
HOB = DH // P    # 32 o-blocks per half
NODT = D2 // 512  # 16 output column tiles
WV_CH = 8        # dq-tiles per streamed Wv chunk
NCH = NT // WV_CH
SCALE = float(1.0 / np.sqrt(512.0))
N_WARM = 24      # dummy warm-up matmuls while the first x chunks stream in

MAX_WAITS = 1  # this walrus build allows one sync-wait per instruction


def _split_sync_waits(nc):
    """Move excess per-instruction sync waits onto preceding no-ops (same
    engine, program order) — semantically identical, keeps walrus happy."""
    for f in nc.m.functions:
        for bb in f.blocks:
            new_insts = []
            for inst in bb.instructions:
                si = inst.sync_info
                if si is not None and si.on_wait and len(si.on_wait) > MAX_WAITS:
                    waits = list(si.on_wait)
                    excess, keep = waits[MAX_WAITS:], waits[:MAX_WAITS]
                    for ci in range(0, len(excess), MAX_WAITS):
                        new_insts.append(mybir.InstNoOp(
                            name=f"{inst.name}-waitsplit{ci}",
                            engine=inst.engine, ins=[], outs=[],
                            sync_info=mybir.SyncInfo(
                                on_wait=excess[ci:ci + MAX_WAITS], on_update=[]),
                            text_hint="waitsplit"))
                    si.on_wait = keep
                new_insts.append(inst)
            bb.instructions = new_insts


def build_nc():
    nc = bass.Bass()
    wq = nc.declare_dram_parameter("wq", [NOB, P, NT, P], BF16, isOutput=False)
    wk = nc.declare_dram_parameter("wk", [NOB, P, NT, P], BF16, isOutput=False)
    wv = nc.declare_dram_parameter("wv", [NODT, P, NT, 512], BF16, isOutput=False)
    qb = nc.declare_dram_parameter("qb", [P, NOB], F32, isOutput=False)
    kb = nc.declare_dram_parameter("kb", [P, NOB], F32, isOutput=False)
    vb = nc.declare_dram_parameter("vb", [D2], F32R, isOutput=False)
    lam = nc.declare_dram_parameter("lam", [S], F32R, isOutput=False)
    ones = nc.declare_dram_parameter("ones", [P], F32R, isOutput=False)
    xTb = nc.declare_dram_parameter("xTb", [DM, S], BF16, isOutput=False)
    out = nc.declare_dram_parameter("out", [S, D2], F32, isOutput=True)

    with tile.TileContext(nc) as tc, ExitStack() as top:
        const = top.enter_context(tc.tile_pool(name="const", bufs=1))
        lam_sb = const.tile([1, S], F32R, name="lam_sb")
        nc.sync.dma_start(lam_sb[:], lam[None, :])
        ones_row = const.tile([1, P], F32R, name="ones_row")
        nc.sync.dma_start(ones_row[:], ones[None, :])
        vb_sb = const.tile([1, NODT, 512], F32R, name="vb_sb")
        nc.sync.dma_start(vb_sb[:], vb.rearrange("(o f) -> o f", f=512)[None])
        xTb_sb = const.tile([P, NT, S], BF16, name="xTb_sb")
        xTb_r = xTb.rearrange("(t p) s -> p t s", p=P)
        for xc in range(8):
            nc.sync.dma_start(xTb_sb[:, xc * (NT // 8):(xc + 1) * (NT // 8), :],
                              xTb_r[:, xc * (NT // 8):(xc + 1) * (NT // 8), :])
        qb_sb = const.tile([P, NOB], F32, name="qb_sb")
        nc.sync.dma_start(qb_sb[:], qb[:])
        kb_sb = const.tile([P, NOB], F32, name="kb_sb")
        nc.sync.dma_start(kb_sb[:], kb[:])
        ident = const.tile([P, P], F32, name="ident")
        make_identity(nc, ident[:])

        lam_bc = const.tile([P, S], F32, name="lam_bc")
        vb_bc = const.tile([P, NODT, 512], F32, name="vb_bc")
        warm_in = const.tile([P, S], BF16, name="warm_in")
        nc.gpsimd.memset(warm_in[:], 0.125)

        # ---- Warm-up: PE work with no x/weight DMA dependency ----
        # lam/vb broadcasts to all 128 partitions via K=1 matmuls, plus dummy
        # bf16 multiplies on an on-chip memset tile. Warms the HAM clock gate
        # (~3.4us of activity) and fills the PE while the x/weight streams
        # land.
        with tc.tile_pool(name="ps_warm", bufs=2, space="PSUM") as ps_warm:
            pt = ps_warm.tile([P, 512], F32, name="lam_ps", tag="wm")
            nc.tensor.matmul(pt[:], ones_row[:], lam_sb[:], start=True, stop=True)
            nc.vector.tensor_copy(out=lam_bc[:], in_=pt[:])
            for odt in range(NODT):
                pt2 = ps_warm.tile([P, 512], F32, name="vb_ps", tag="wm")
                nc.tensor.matmul(pt2[:], ones_row[:], vb_sb[:, odt, :],
                                 start=True, stop=True)
                nc.vector.tensor_copy(out=vb_bc[:, odt, :], in_=pt2[:])
            for w in range(N_WARM):
                ptw = ps_warm.tile([P, 512], F32, name="warm_ps", tag="wm")
                nc.tensor.matmul(ptw[:], warm_in[:, :P], warm_in[:],
                                 start=True, stop=True)

        e_sb = const.tile([P, 2, NQB, S], F32, name="e_sb")
        d_sb = const.tile([P, 2, NQB], F32, name="d_sb")
        r_sb = const.tile([P, 2, NQB], F32, name="r_sb")
        P_sb = const.tile([P, NQB, S], F32, name="P_sb")
        PT_sb = const.tile([P, S // P, S], BF16, name="PT_sb")
        wv0 = const.tile([P, WV_CH, 512], BF16, name="wv0")

        # ---- Phase A: fused q/k projections + score accumulation ----
        with ExitStack() as phA:
            wqk = phA.enter_context(tc.tile_pool(name="wqk", bufs=5))
            qksb = phA.enter_context(tc.tile_pool(name="qksb", bufs=6))
            ps_proj = phA.enter_context(tc.tile_pool(name="ps_proj", bufs=3, space="PSUM"))
            ps_scores = phA.enter_context(tc.tile_pool(name="ps_scores", bufs=5, space="PSUM"))

            for h in range(2):
                sc_tiles = [ps_scores.tile([P, S], F32, name=f"sc_{h}_{qbk}", tag="sc")
                            for qbk in range(NQB)]
                prev_qk = None

                def emit_scores(i, q_sb, k_sb):
                    for qbk in range(NQB):
                        nc.tensor.matmul(sc_tiles[qbk][:],
                                         q_sb[:, qbk * P:(qbk + 1) * P], k_sb[:],
                                         start=(i == 0), stop=(i == HOB - 1))

                for i in range(HOB):
                    ob = h * HOB + i
                    if h == 1 and i == HOB - 8:
                        # prefetch the first Wv chunk so the PE never idles at
                        # the A->B phase boundary
                        nc.sync.dma_start(wv0[:], wv[0][:, 0:WV_CH, :])
                    pq = ps_proj.tile([P, S], F32, name="pq", tag="pp")
                    for cw in range(2):
                        wt = wqk.tile([P, NT // 2, P], BF16, name="wt_q", tag="w")
                        nc.sync.dma_start(wt[:], wq[ob][:, cw * (NT // 2):(cw + 1) * (NT // 2), :])
                        for tt in range(NT // 2):
                            t = cw * (NT // 2) + tt
                            nc.tensor.matmul(pq[:], wt[:, tt, :], xTb_sb[:, t, :],
                                             start=(t == 0), stop=(t == NT - 1))
                    q_sb = qksb.tile([P, S], BF16, name="q_sb", tag="qk")
                    nc.vector.tensor_scalar(q_sb[:], pq[:], qb_sb[:, ob:ob + 1], SCALE,
                                            mybir.AluOpType.add, mybir.AluOpType.mult)
                    pk = ps_proj.tile([P, S], F32, name="pk", tag="pp")
                    for cw in range(2):
                        wtk = wqk.tile([P, NT // 2, P], BF16, name="wt_k", tag="w")
                        nc.sync.dma_start(wtk[:], wk[ob][:, cw * (NT // 2):(cw + 1) * (NT // 2), :])
                        for tt in range(NT // 2):
                            t = cw * (NT // 2) + tt
                            nc.tensor.matmul(pk[:], wtk[:, tt, :], xTb_sb[:, t, :],
                                             start=(t == 0), stop=(t == NT - 1))
                    if prev_qk is not None:
                        emit_scores(i - 1, *prev_qk)
                    k_sb = qksb.tile([P, S], BF16, name="k_sb", tag="qk")
                    nc.vector.tensor_scalar(k_sb[:], pk[:], kb_sb[:, ob:ob + 1], None,
                                            mybir.AluOpType.add)
                    prev_qk = (q_sb, k_sb)
                emit_scores(HOB - 1, *prev_qk)
                for qbk in range(NQB):
                    nc.scalar.activation(e_sb[:, h, qbk, :], sc_tiles[qbk][:],
                                         mybir.ActivationFunctionType.Exp,
                                         accum_out=d_sb[:, h, qbk:qbk + 1])

        # ---- Phase B+C: combine + transpose overlapped with v projection/AV ----
        nc.vector.reciprocal(r_sb[:, :, :], d_sb[:, :, :])
        with ExitStack() as phC:
            cmb = phC.enter_context(tc.tile_pool(name="cmb", bufs=2))
            wvp = phC.enter_context(tc.tile_pool(name="wvp", bufs=3))
            vsb = phC.enter_context(tc.tile_pool(name="vsb", bufs=2))
            osb = phC.enter_context(tc.tile_pool(name="osb", bufs=4))
            ps_tr = phC.enter_context(tc.tile_pool(name="ps_tr", bufs=2, space="PSUM"))
            ps_vp = phC.enter_context(tc.tile_pool(name="ps_vp", bufs=4, space="PSUM"))
            ps_av = phC.enter_context(tc.tile_pool(name="ps_av", bufs=2, space="PSUM"))

            def combine_and_transpose():
                for qbk in range(NQB):
                    tmp = cmb.tile([P, S], F32, name="tmp", tag="tmp")
                    nc.vector.tensor_tensor(tmp[:], e_sb[:, 1, qbk, :], lam_bc[:],
                                            mybir.AluOpType.mult)
                    nc.vector.tensor_scalar(tmp[:], tmp[:], r_sb[:, 1, qbk:qbk + 1], None,
                                            mybir.AluOpType.mult)
                    nc.vector.tensor_scalar(P_sb[:, qbk, :], e_sb[:, 0, qbk, :],
                                            r_sb[:, 0, qbk:qbk + 1], None,
                                            mybir.AluOpType.mult)
                    nc.vector.tensor_tensor(P_sb[:, qbk, :], P_sb[:, qbk, :], tmp[:],
                                            mybir.AluOpType.subtract)
                for qbk in range(NQB):
                    for kbk in range(S // P):
                        pt2 = ps_tr.tile([P, P], F32, name="pt2", tag="pt")
                        nc.tensor.transpose(pt2[:], P_sb[:, qbk, kbk * P:(kbk + 1) * P],
                                            ident[:])
                        nc.vector.tensor_copy(out=PT_sb[:, kbk, qbk * P:(qbk + 1) * P],
                                              in_=pt2[:])

            v_tiles = [None] * NODT

            def do_av(odt):
                for qbk in range(NQB):
                    pav = ps_av.tile([P, 512], F32, name="pav", tag="av")
                    for kbk in range(S // P):
                        nc.tensor.matmul(pav[:],
                                         PT_sb[:, kbk, qbk * P:(qbk + 1) * P],
                                         v_tiles[odt][:, kbk, :],
                                         start=(kbk == 0), stop=(kbk == S // P - 1))
                    o_st = osb.tile([P, 512], F32, name="o_st", tag="o")
                    nc.vector.tensor_copy(out=o_st[:], in_=pav[:])
                    nc.sync.dma_start(out[qbk * P:(qbk + 1) * P, odt * 512:(odt + 1) * 512],
                                      o_st[:])

            for odt in range(NODT):
                pv = [ps_vp.tile([P, 512], F32, name=f"pv{sb}", tag="vp")
                      for sb in range(NQB)]
                for c in range(NCH):
                    if odt == 0 and c == 0:
                        wvt = wv0
                    else:
                        wvt = wvp.tile([P, WV_CH, 512], BF16, name="wvt", tag="wv")
                        nc.sync.dma_start(wvt[:], wv[odt][:, c * WV_CH:(c + 1) * WV_CH, :])
                    for sb in range(NQB):
                        for tt in range(WV_CH):
                            t = c * WV_CH + tt
                            nc.tensor.matmul(pv[sb][:], xTb_sb[:, t, sb * P:(sb + 1) * P],
                                             wvt[:, tt, :],
                                             start=(t == 0), stop=(t == NT - 1))
                v_t = vsb.tile([P, S // P, 512], BF16, name="v_t", tag="v")
                for sb in range(NQB):
                    nc.vector.tensor_tensor(v_t[:, sb, :], pv[sb][:], vb_bc[:, odt, :],
                                            mybir.AluOpType.add)
                v_tiles[odt] = v_t
                if odt == 0:
                    # PE chews on v-proj(0) while DVE does the combine and the
                    # transposes queue up behind it — hides the softmax tail.
                    combine_and_transpose()
                if odt >= 1:
                    do_av(odt - 1)
                    v_tiles[odt - 1] = None
            do_av(NODT - 1)

    _split_sync_waits(nc)
    return nc


def pack_shared(wq_w, wq_b, wk_w, wk_b, wv_w, wv_b,
                lambda_q1, lambda_k1, lambda_q2, lambda_k2):
    lam = (np.exp(lambda_q1 * lambda_k1) - np.exp(lambda_q2 * lambda_k2)
           + np.float32(0.8)).astype(np.float32)
    return {
        "wq": np.ascontiguousarray(wq_w.reshape(NOB, P, NT, P).transpose(0, 3, 2, 1)).astype(ml_dtypes.bfloat16),
        "wk": np.ascontiguousarray(wk_w.reshape(NOB, P, NT, P).transpose(0, 3, 2, 1)).astype(ml_dtypes.bfloat16),
        "wv": np.ascontiguousarray(wv_w.reshape(NODT, 512, NT, P).transpose(0, 3, 2, 1)).astype(ml_dtypes.bfloat16),
        "qb": np.ascontiguousarray(wq_b.reshape(NOB, P).T),
        "kb": np.ascontiguousarray(wk_b.reshape(NOB, P).T),
        "vb": np.ascontiguousarray(wv_b),
        "lam": lam,
        "ones": np.ones(P, np.float32),
    }


def make_in_maps(x, wq_w, wq_b, wk_w, wk_b, wv_w, wv_b,
                 lambda_q1, lambda_k1, lambda_q2, lambda_k2):
    shared = pack_shared(wq_w, wq_b, wk_w, wk_b, wv_w, wv_b,
                         lambda_q1, lambda_k1, lambda_q2, lambda_k2)
    return [{**shared,
             "xTb": np.ascontiguousarray(x[b].T).astype(ml_dtypes.bfloat16)}
            for b in range(B)]


_NC_CACHE = None


def get_nc():
    global _NC_CACHE
    if _NC_CACHE is None:
        _NC_CACHE = build_nc()
    return _NC_CACHE


def kernel(x, wq_w, wq_b, wk_w, wk_b, wv_w, wv_b,
           lambda_q1, lambda_k1, lambda_q2, lambda_k2):
    args = [np.asarray(a, dtype=np.float32) for a in
            (x, wq_w, wq_b, wk_w, wk_b, wv_w, wv_b,
             lambda_q1, lambda_k1, lambda_q2, lambda_k2)]
    nc = get_nc()
    in_maps = make_in_maps(*args)
    res = run_bass_kernel_spmd(nc, in_maps, list(range(B)))
    return np.stack([res.results[b]["out"] for b in range(B)]).astype(np.float32)
